# revision 4
# baseline (speedup 1.0000x reference)
"""Trainium2 Bass kernel for nn_CompletePatchReadout.

Reference computation:
  xb  = rearrange(x, 'B t p f -> B p (t f)')             # [B, P, D]
  out = einsum('bpd,pdnh->bpnh', xb, W) + b[None]        # [B, P, MAXC, H]
  buf = zeros(B, N+1, H); buf[:, node_map.flat] = out    # scatter (permutation)
  y   = rearrange(buf[:, :N], 'B n h -> (n B) h')

The kernel is DMA-bound on streaming W: the per-core DMA fabric is 16
engines x ~24.4 GB/s ~= 390 GB/s, and the fp16 baseline ran it at that
roofline for 96 of 113 us.  Levers, in order of impact:

1. Ragged trim.  Only counts[p] in [20,48] of the MAXC=48 node columns
   per patch are real (node_map pads with the dummy node N); the padded
   columns' outputs are discarded.  Mean count is 34/48, so skipping the
   dead columns drops ~29% of W (and y) bytes.  To keep one SPMD program
   across the 8 cores, patches are sorted by count and dealt round-robin
   (rank r -> core r%8, slot r//8): slot j is padded to the max count
   across cores, which by the sort is the count at rank 8j.  Padding
   waste is ~2.3%.

2. fp8e3 (E3M4) W, and fp8e3 for the t>=6 half of x.  The tensor engine
   accepts mixed fp16/fp8e3 operands; E3M4's 4 mantissa bits give ~1.3%
   rms rel err per operand stream on this GEMM (vs ~2.5% for e4m3).
   W fp8 + half-fp8 x measures 1.62e-2 rms rel err, inside the 2e-2
   gate.  Quantization scales (global u for x, per-patch s_p for W)
   make every stream span E3M4's normal range; PSUM then holds
   u*s_p*(x@W), bias is pre-scaled by u*s_p on the host, and the
   descale by 1/(u*s_p) folds into the host-side unshard.

3. DMA-trigger economy.  Each DMA_DIRECT2D costs ~630ns on its issuing
   engine, and a trigger that waits blocks every trigger behind it on
   the same queue.  The whole per-core W stream is only ~80KB/partition,
   so all 16 slot tiles are SBUF-resident (bufs=1, no ring reuse): the
   sync queue issues [W0, W1, const, W2..W15] back-to-back with zero
   waits.  The x chunks and y writebacks trigger from the Activation
   engine's HWDGE queue instead, so y triggers (which wait on DVE
   copies) never stall the W stream.  y returns as fp16 (halves
   writeback bytes; ~1e-4 rel).

Slots are processed in pairs packed into the two 64-partition halves of
PSUM (patch A -> partitions 0-63, B -> 64-127).  A pair's column space
[0, M_A) is split into blocks of <=42 nodes (42*12*4B = 2016B <= one
2KB PSUM bank).  The node_map scatter stays on the host (a pure
permutation of output rows).
"""

import os

import numpy as np
import ml_dtypes

from concourse import bacc
import concourse.mybir as mybir
from concourse import bass_utils
from concourse.tile import TileContext

# Problem shapes (hardcoded per harness contract)
B, T, P, F, H, MAXC = 64, 12, 128, 128, 12, 48
D = T * F            # 1536
N_NODES = 4356       # sum of ragged patch counts in the reference
NCORES = 8
NPOS = P // NCORES   # 16 patch slots per core
NPAIR = NPOS // 2    # 8 slot pairs per core
MAX_PSUM_NODES = 42  # 42*H*4B = 2016B fits one 2KB PSUM bank
TH = T // 2          # x timestep split: t<TH fp16, t>=TH fp8e3

F32 = mybir.dt.float32
F16 = mybir.dt.float16
F8 = mybir.dt.float8e3          # E3M4
NP_F8 = ml_dtypes.float8_e3m4
E3M4_MAX = 15.0                 # quant target just under E3M4 max (15.5)

# Populated by kernel() after each run (test.py reads this for profiling).
LAST_RESULTS = None


def _make_schedule(node_map):
    """Global (core-invariant) ragged schedule derived from node_map."""
    counts = (np.asarray(node_map) != N_NODES).sum(axis=1).astype(np.int64)
    counts = np.clip(counts, 1, MAXC)
    order = np.argsort(-counts, kind="stable")       # rank -> patch id
    # rank r -> (core r%8, slot r//8); padded slot size = count at rank 8j.
    M = [int(counts[order[8 * j]]) for j in range(NPOS)]
    pairs = []
    oy = 0
    for q in range(NPAIR):
        MA = M[2 * q]
        if MA <= MAX_PSUM_NODES:
            blocks = [(0, MA)]
        else:
            h1 = (MA + 1) // 2
            blocks = [(0, h1), (h1, MA - h1)]
        bl = []
        for o, m in blocks:
            bl.append((o, m, oy))
            oy += m * H
        pairs.append(bl)
    return {"counts": counts, "order": order, "M": M, "pairs": pairs,
            "toty": oy}


def _build_bass(sched):
    M, pairs, TOTY = sched["M"], sched["pairs"], sched["toty"]
    CH = [M[j] * H for j in range(NPOS)]
    nc = bacc.Bacc("TRN2", target_bir_lowering=False, debug=False,
                   num_devices=NCORES)

    xh_d = nc.dram_tensor("xh", [F, NPOS * TH * B], F16, kind="ExternalInput")
    xl_d = nc.dram_tensor("xl", [F, NPOS * TH * B], F8, kind="ExternalInput")
    w_d = [nc.dram_tensor(f"W{j}", [F, T * CH[j]], F8, kind="ExternalInput")
           for j in range(NPOS)]
    const_d = nc.dram_tensor("const", [2, 128 + TOTY], F16,
                             kind="ExternalInput")
    y_d = nc.dram_tensor("y", [128, TOTY], F16, kind="ExternalOutput")

    XC = 2 * TH * B      # x columns per pair chunk (per precision half)

    with TileContext(nc) as tc:
        with (
            tc.tile_pool(name="cpool", bufs=1) as cpool,
            tc.tile_pool(name="wpool", bufs=1) as wpool,
            tc.tile_pool(name="opool", bufs=4) as opool,
            tc.tile_pool(name="psum", bufs=3, space="PSUM") as pspool,
        ):
            const_sb = cpool.tile([2, 128 + TOTY], F16)
            xh_sb = cpool.tile([F, NPOS * TH * B], F16)
            xl_sb = cpool.tile([F, NPOS * TH * B], F8)

            def x_chunk_dma(q):
                c0 = q * XC
                nc.scalar.dma_start(out=xh_sb[:, c0:c0 + XC],
                                    in_=xh_d[:, c0:c0 + XC])
                nc.scalar.dma_start(out=xl_sb[:, c0:c0 + XC],
                                    in_=xl_d[:, c0:c0 + XC])

            for q in range(NPAIR):
                x_chunk_dma(q)

            # All 16 W slot tiles are SBUF-resident (no ring reuse), so the
            # sync queue's W triggers issue back-to-back with no waits.
            # Order: W0, W1, const, W2.. so pair 0's stream starts first
            # (const is tiny and lands during W0's transfer).
            w_sb = [wpool.tile([F, T * CH[j]], F8, name=f"w{j}", tag=f"w{j}",
                               bufs=1) for j in range(NPOS)]
            nc.sync.dma_start(out=w_sb[0][:], in_=w_d[0][:])
            nc.sync.dma_start(out=w_sb[1][:], in_=w_d[1][:])
            nc.sync.dma_start(out=const_sb[:], in_=const_d[:])
            for j in range(2, NPOS):
                nc.sync.dma_start(out=w_sb[j][:], in_=w_d[j][:])

            # Observer matmuls: absorb DMA semaphores into throwaway PE ops
            # so the real matmuls stay wait-lean.
            scratch = pspool.tile([64, 64], F32, name="scratch", bufs=1)
            nc.tensor.matmul(
                scratch[:], const_sb[:, 0:64], const_sb[:, 64:128],
                start=True, stop=True, skip_group_check=True,
            )

            def observe_x(q):
                c0 = q * XC
                nc.tensor.matmul(
                    scratch[:, 0:16], xh_sb[:, c0:c0 + 64],
                    xh_sb[:, c0:c0 + 16],
                    start=True, stop=True, skip_group_check=True,
                )
                nc.tensor.matmul(
                    scratch[:, 0:16], xl_sb[:, c0:c0 + 64],
                    xl_sb[:, c0:c0 + 16],
                    start=True, stop=True, skip_group_check=True,
                )

            observe_x(0)
            observe_x(1)

            sel_ap = const_sb[:, 0:128]          # [2, 128] column selector

            def bias_ap(oy, m):
                return const_sb[:, 128 + oy: 128 + oy + m * H]

            def x_ap(j, t):
                if t < TH:
                    return xh_sb[:, (j * TH + t) * B: (j * TH + t + 1) * B]
                tl = t - TH
                return xl_sb[:, (j * TH + tl) * B: (j * TH + tl + 1) * B]

            y_off = 0
            for q in range(NPAIR):
                ja, jb = 2 * q, 2 * q + 1
                MB = M[jb]
                CHA, CHB = CH[ja], CH[jb]
                blocks = pairs[q]
                wa, wb = w_sb[ja], w_sb[jb]
                ps = []
                for k, (o, m, oy) in enumerate(blocks):
                    pst = pspool.tile([128, m * H], F32, name=f"ps{k}")
                    # One K=2 selector matmul writes both patches' bias rows
                    # and is the bank's single start=True.
                    nc.tensor.matmul(pst[:], sel_ap, bias_ap(oy, m),
                                     start=True, stop=False)
                    ps.append(pst)

                for t in range(T):
                    la, lb = x_ap(ja, t), x_ap(jb, t)
                    last = t == T - 1
                    for k, (o, m, oy) in enumerate(blocks):
                        mB = min(o + m, MB) - o
                        # A -> PE columns 0-63, B -> 64-127 (tile_position
                        # inferred from out.base_partition()).
                        nc.tensor.matmul(
                            ps[k][0:64], la,
                            wa[:, t * CHA + o * H: t * CHA + (o + m) * H],
                            start=False, stop=last and mB <= 0,
                        )
                        if mB > 0:
                            nc.tensor.matmul(
                                ps[k][64:128, 0:mB * H], lb,
                                wb[:, t * CHB + o * H:
                                   t * CHB + (o + mB) * H],
                                start=False, stop=last,
                            )

                if q + 2 < NPAIR:
                    observe_x(q + 2)

                # Evacuate each PSUM bank with ONE full-tile DVE copy (a
                # partial read would race PE writes to the same bank), pack
                # the pair's blocks into one fp16 tile, one y trigger on the
                # Activation HWDGE queue (so its copy-wait can't stall W
                # triggers on the sync queue).
                st = opool.tile([128, CHA], F16, name="st", tag="st")
                for k, (o, m, oy) in enumerate(blocks):
                    nc.vector.tensor_copy(st[:, o * H:(o + m) * H], ps[k][:])
                nc.scalar.dma_start(out=y_d[:, y_off:y_off + CHA], in_=st[:])
                y_off += CHA

    nc.compile()  # bacc passes: split sync waits to the 1-per-inst HW limit
    return nc


def _make_in_maps(inputs, sched):
    x = np.asarray(inputs["x"], dtype=np.float32)     # [B, T, P, F]
    W = np.asarray(inputs["W"], dtype=np.float32)     # [P, D, MAXC, H]
    b = np.asarray(inputs["b"], dtype=np.float32)     # [P, MAXC, H]
    counts, order = sched["counts"], sched["order"]
    M, pairs, TOTY = sched["M"], sched["pairs"], sched["toty"]

    Wt = W.reshape(P, T, F, MAXC, H)
    axmax = float(np.abs(x).max())
    u = E3M4_MAX / axmax if axmax > 0 else 1.0
    wscale = np.ones(P, np.float64)

    in_maps = []
    for c in range(NCORES):
        im = {}
        xh = np.zeros((F, NPOS * TH * B), np.float16)
        xl = np.zeros((F, NPOS * TH * B), NP_F8)
        const = np.zeros((2, 128 + TOTY), np.float16)
        const[0, 0:64] = 1.0
        const[1, 64:128] = 1.0
        for j in range(NPOS):
            p = int(order[8 * j + c])
            cp = int(counts[p])
            Mj = M[j]
            wp = Wt[p][:, :, :cp, :]                  # [T, F, cp, H]
            amax = float(np.abs(wp).max())
            s = E3M4_MAX / amax if amax > 0 else 1.0
            wscale[p] = s
            q8 = np.zeros((F, T, Mj, H), NP_F8)
            q8[:, :, :cp, :] = (wp.transpose(1, 0, 2, 3) * s).astype(NP_F8)
            im[f"W{j}"] = np.ascontiguousarray(q8.reshape(F, T * Mj * H))
            xs = x[:, :, p, :] * u                    # [B, T, F]
            xh[:, j * TH * B:(j + 1) * TH * B] = (
                xs[:, :TH].transpose(2, 1, 0).reshape(F, TH * B)
            )
            xl[:, j * TH * B:(j + 1) * TH * B] = (
                xs[:, TH:].transpose(2, 1, 0).reshape(F, TH * B)
            )
        im["xh"], im["xl"] = xh, xl
        for q in range(NPAIR):
            pa = int(order[8 * (2 * q) + c])
            pb = int(order[8 * (2 * q + 1) + c])
            ca, cb = int(counts[pa]), int(counts[pb])
            # bias pre-scaled by u*s so PSUM is uniformly u*s*(x@W + b/(us))
            for o, m, oy in pairs[q]:
                blkA = b[pa, o:o + m, :] * (u * wscale[pa])
                blkA[max(ca - o, 0):] = 0             # zero padded slots
                const[0, 128 + oy: 128 + oy + m * H] = blkA.reshape(-1)
                blkB = b[pb, o:o + m, :] * (u * wscale[pb])
                blkB[max(cb - o, 0):] = 0
                const[1, 128 + oy: 128 + oy + m * H] = blkB.reshape(-1)
        im["const"] = const
        in_maps.append(im)
    sched["u"] = u
    sched["wscale"] = wscale
    return in_maps


def _run(nc, in_maps, trace=False):
    return bass_utils.run_bass_kernel_spmd(
        nc, in_maps, core_ids=list(range(NCORES)), trace=trace
    )


def _postprocess(results, node_map, sched):
    counts, order = sched["counts"], sched["order"]
    M, pairs = sched["M"], sched["pairs"]
    u, wscale = sched["u"], sched["wscale"]
    node_map = np.asarray(node_map)

    inv = np.empty(P, np.int64)
    inv[order] = np.arange(P)                         # patch -> rank

    # Host-side unshard: descale by 1/(u*s_p), apply the node_map
    # permutation (scatter) and the final 'B n h -> (n B) h' rearrange.
    buf = np.zeros((B, N_NODES + 1, H), dtype=np.float32)
    for p in range(P):
        r = int(inv[p])
        c, j = r % NCORES, r // NCORES
        y = results[c]["y"]                           # [128, TOTY] fp16
        cp = int(counts[p])
        q, half = j // 2, j % 2
        rows = slice(0, 64) if half == 0 else slice(64, 128)
        Mj = M[j]
        segs = []
        for o, m, oy in pairs[q]:
            mv = min(o + m, Mj) - o
            if mv > 0:
                segs.append(y[rows, oy: oy + mv * H])
        yp = np.concatenate(segs, axis=1)[:, :cp * H].astype(np.float32)
        yp *= 1.0 / (u * wscale[p])
        buf[:, node_map[p, :cp], :] = yp.reshape(B, cp, H)
    out = buf[:, :N_NODES, :]
    return np.ascontiguousarray(out.transpose(1, 0, 2)).reshape(N_NODES * B, H)


def kernel(**inputs) -> np.ndarray:
    global LAST_RESULTS

    node_map = np.asarray(inputs["node_map"])
    sched = _make_schedule(node_map)
    in_maps = _make_in_maps(inputs, sched)
    nc = _build_bass(sched)
    trace = os.environ.get("KERNEL_TRACE") == "1"
    res = _run(nc, in_maps, trace=trace)
    LAST_RESULTS = res
    return _postprocess(res.results, node_map, sched)
